# revision 3
# baseline (speedup 1.0000x reference)
"""Causal multi-head attention (B=4, S=2048, D=1024, H=16, Dh=64) on 8
Trainium2 NeuronCores.

Sharding: hybrid batch x head-half.  Core c handles batch b = c//2 and heads
[8*(c%2), 8*(c%2)+8).  Weights are column-sharded (Wq/Wk/Wv) / row-sharded
(Wo); each core computes a partial output projection summed on the host, plus
its 8 heads' attention probabilities.

Device kernel (per core, identical program, different data):
  phase A: QT/KT = (x @ Wq/Wk).T and V = x @ Wv from a host-pretransposed
           xT, all in fp32r (TF32-class) matmuls.
  phase B: per head-pair and 512-wide i-column: scoresT[j,i] = K Q^T (only
           causal j-blocks), exp via ScalarE (scale=1/8 folded in), causal
           masking via mask-tile multiplies, softmax denominator via a
           ones-vector matmul accumulated in PSUM, reciprocal + ones-matmul
           broadcast, in-place normalize, DMA of probsT, and probs @ V
           accumulated over j-blocks.
  phase C: out_partial = attn @ Wo.

attn_probs is produced TRANSPOSED per head ([j, i]); the host transposes.
Upper-triangle j-blocks are never written: PJRT output buffers are donated
zeros, so they come back as exact 0 (matching the causal mask).
"""

import numpy as np

import concourse.bass as bass
import concourse.tile as tile
from concourse import mybir
from concourse.bass_utils import run_bass_kernel_spmd

f32 = mybir.dt.float32
f32r = mybir.dt.float32r

B, S, D, H, Dh = 4, 2048, 1024, 16, 64
NCH = D // 128      # 8 contraction chunks over d
NSB = S // 128      # 16 s-blocks
NCI = S // 512      # 4 i-column groups
NJB = S // 128      # 16 j-blocks
HPC = 8             # heads per core
NHP = HPC // 2      # head pairs per core


def _split_multi_waits(nc, limit=1):
    """This container's walrus accepts only ONE sync wait per instruction.
    Rewrite any instruction with more into single-wait NoOps (same engine,
    program order right before it) + the instruction keeping the last wait."""
    k = 0
    for f in nc.m.functions:
        for blk in f.blocks:
            old = blk.instructions
            new = []
            for inst in old:
                si = inst.sync_info
                if si is not None and si.on_wait and len(si.on_wait) > limit:
                    waits = list(si.on_wait)
                    for w in waits[:-limit]:
                        k += 1
                        nop = mybir.InstNoOp(name=f"WSPLIT-{k}", ins=[], outs=[])
                        nop.engine = inst.engine
                        nop.sync_info = mybir.SyncInfo(on_wait=[w], on_update=[])
                        new.append(nop)
                    inst.sync_info = mybir.SyncInfo(
                        on_wait=waits[-limit:], on_update=list(si.on_update)
                    )
                new.append(inst)
            blk.instructions = new
    return k


def _build():
    nc = bass.Bass()
    xt_d = nc.dram_tensor("xt", [D, S], f32, kind="ExternalInput")
    wq_d = nc.dram_tensor("wq", [D, 512], f32, kind="ExternalInput")
    wk_d = nc.dram_tensor("wk", [D, 512], f32, kind="ExternalInput")
    wv_d = nc.dram_tensor("wv", [D, 512], f32, kind="ExternalInput")
    wo_d = nc.dram_tensor("wo", [512, D], f32, kind="ExternalInput")
    masks_d = nc.dram_tensor("masks", [4, 128, 512], f32, kind="ExternalInput")
    probs_d = nc.dram_tensor("probsT", [HPC, S, S], f32, kind="ExternalOutput")
    outp_d = nc.dram_tensor("outp", [S, D], f32, kind="ExternalOutput")

    with tile.TileContext(nc) as tc:
        # ---- long-lived pools first (stack allocator: longest-lived lowest) ----
        with (
            tc.tile_pool(name="qt", bufs=1) as qt_pool,
            tc.tile_pool(name="kt", bufs=1) as kt_pool,
            tc.tile_pool(name="v", bufs=1) as v_pool,
            tc.tile_pool(name="misc", bufs=1) as misc_pool,
        ):
            # [128 (dh of head pair), 2048 (s)] fp32r
            qt = [qt_pool.tile([128, S], f32r, tag=f"qt{i}", name=f"qt{i}") for i in range(NHP)]
            kt = [kt_pool.tile([128, S], f32r, tag=f"kt{i}", name=f"kt{i}") for i in range(NHP)]
            # [128 (s-block), 512 (dh of 8 heads)] fp32r
            v = [v_pool.tile([128, 512], f32r, tag=f"v{i}", name=f"v{i}") for i in range(NSB)]

            masks = misc_pool.tile([128, 4, 512], f32, tag="masks", name="masks")
            ones_col = misc_pool.tile([128, 1], f32r, tag="ones_col", name="ones_col")
            ones_row = misc_pool.tile([1, 128], f32r, tag="ones_row", name="ones_row")
            ones_row_f = misc_pool.tile([1, 128], f32, tag="ones_row_f", name="ones_row_f")
            ones_f = misc_pool.tile([128, 1], f32, tag="ones_f", name="ones_f")

            nc.sync.dma_start(out=masks, in_=masks_d.rearrange("k p n -> p k n"))
            nc.vector.memset(ones_f, 1.0)
            nc.vector.memset(ones_row_f, 1.0)
            nc.vector.tensor_copy(ones_col, ones_f)
            nc.vector.tensor_copy(ones_row, ones_row_f)

            # ---------------- phase A: projections (sliced over s) ----------------
            with (
                tc.tile_pool(name="wabc", bufs=1) as w_pool,
                tc.tile_pool(name="psA", bufs=2, space="PSUM") as psA,
            ):
                wq = [w_pool.tile([128, 512], f32r, tag=f"wq{i}", name=f"wq{i}") for i in range(NCH)]
                wk = [w_pool.tile([128, 512], f32r, tag=f"wk{i}", name=f"wk{i}") for i in range(NCH)]
                wv = [w_pool.tile([128, 512], f32r, tag=f"wv{i}", name=f"wv{i}") for i in range(NCH)]
                with tc.tile_pool(name="wstage", bufs=3) as wst:
                    for ch in range(NCH):
                        for dst, dram in ((wq, wq_d), (wk, wk_d), (wv, wv_d)):
                            t = wst.tile([128, 512], f32, tag="ws", name=f"ws{ch}_{id(dram)}")
                            nc.sync.dma_start(out=t, in_=dram[ch * 128:(ch + 1) * 128, :])
                            nc.scalar.copy(out=dst[ch], in_=t)

                with tc.tile_pool(name="xsc", bufs=1) as xsc_pool, tc.tile_pool(name="xstage", bufs=2) as xst:
                    for sc in range(NCI):
                        xt = []
                        for ch in range(NCH):
                            t = xst.tile([128, 512], f32, tag="xs", name=f"xs{sc}_{ch}")
                            nc.sync.dma_start(
                                out=t, in_=xt_d[ch * 128:(ch + 1) * 128, sc * 512:(sc + 1) * 512]
                            )
                            xr = xsc_pool.tile([128, 512], f32r, tag=f"xr{ch}", name=f"xr{sc}_{ch}", bufs=1)
                            nc.vector.tensor_copy(xr, t)
                            xt.append(xr)
                        # QT / KT columns for this s-slice
                        for dst, w in ((qt, wq), (kt, wk)):
                            for hp in range(NHP):
                                ps = psA.tile([128, 512], f32, tag="psA", name=f"pA{sc}_{id(w)}_{hp}")
                                for ch in range(NCH):
                                    nc.tensor.matmul(
                                        ps,
                                        w[ch][:, hp * 128:(hp + 1) * 128],
                                        xt[ch],
                                        start=(ch == 0),
                                        stop=(ch == NCH - 1),
                                    )
                                if hp % 2 == 0:
                                    nc.scalar.copy(out=dst[hp][:, sc * 512:(sc + 1) * 512], in_=ps)
                                else:
                                    nc.vector.tensor_copy(dst[hp][:, sc * 512:(sc + 1) * 512], ps)
                        # V rows for this s-slice
                        for k in range(4):
                            sb = 4 * sc + k
                            ps = psA.tile([128, 512], f32, tag="psA", name=f"pV{sb}")
                            for ch in range(NCH):
                                nc.tensor.matmul(
                                    ps,
                                    xt[ch][:, k * 128:(k + 1) * 128],
                                    wv[ch],
                                    start=(ch == 0),
                                    stop=(ch == NCH - 1),
                                )
                            if k % 2 == 0:
                                nc.scalar.copy(out=v[sb], in_=ps)
                            else:
                                nc.vector.tensor_copy(v[sb], ps)

            # attn^T per head pair: [128 (dh), 2048 (i)] fp32r
            with tc.tile_pool(name="a8", bufs=1) as a8_pool:
                a8 = [a8_pool.tile([128, S], f32r, tag=f"a8{i}", name=f"a8{i}") for i in range(NHP)]

                # ---------------- phase B: attention ----------------
                # Exp tiles go out UNNORMALIZED (host divides by colsum); attnV
                # accumulates unnormalized exp and the 1/colsum scale is applied
                # per-column during PSUM evacuation via a broadcast tile.
                with (
                    tc.tile_pool(name="et", bufs=6) as et_pool,
                    tc.tile_pool(name="recip", bufs=4) as recip_pool,
                    tc.tile_pool(name="bcs", bufs=4) as bcs_pool,
                    tc.tile_pool(name="score", bufs=2, space="PSUM") as score_ps,
                    tc.tile_pool(name="cs", bufs=2, space="PSUM") as cs_ps,
                    tc.tile_pool(name="bc", bufs=2, space="PSUM") as bc_ps,
                    tc.tile_pool(name="at", bufs=2, space="PSUM") as at_ps,
                ):
                    for hp in range(NHP):
                        for ci in range(NCI):
                            njb = 4 * ci + 4
                            cs = [cs_ps.tile([1, 512], f32, tag="cs", name=f"cs{hp}_{ci}_{h}") for h in range(2)]
                            p_at = [
                                at_ps.tile([128, 512], f32, tag="at", name=f"at{hp}_{ci}_{h}")
                                for h in range(2)
                            ]
                            for jb in range(njb):
                                diag_k = jb - 4 * ci
                                for h in range(2):
                                    off = h * 64
                                    ps = score_ps.tile([128, 512], f32, tag="score", name=f"s{hp}_{ci}_{jb}_{h}")
                                    nc.tensor.matmul(
                                        ps,
                                        kt[hp][off:off + 64, jb * 128:(jb + 1) * 128],
                                        qt[hp][off:off + 64, ci * 512:(ci + 1) * 512],
                                        start=True,
                                        stop=True,
                                    )
                                    et = et_pool.tile([128, 512], f32r, tag="et", name=f"et{hp}_{ci}_{jb}_{h}")
                                    nc.scalar.activation(
                                        out=et, in_=ps,
                                        func=mybir.ActivationFunctionType.Exp,
                                        scale=0.125,
                                    )
                                    if diag_k >= 0:
                                        nc.vector.tensor_tensor(
                                            out=et, in0=et, in1=masks[:, diag_k, :],
                                            op=mybir.AluOpType.mult,
                                        )
                                    nc.tensor.matmul(
                                        cs[h], ones_col, et,
                                        start=(jb == 0), stop=(jb == njb - 1),
                                    )
                                    nc.sync.dma_start(
                                        out=probs_d[
                                            2 * hp + h,
                                            jb * 128:(jb + 1) * 128,
                                            ci * 512:(ci + 1) * 512,
                                        ],
                                        in_=et.bitcast(f32),
                                    )
                                    nc.tensor.matmul(
                                        p_at[h],
                                        v[jb][:, hp * 128:(hp + 1) * 128],
                                        et,
                                        start=(jb == 0),
                                        stop=(jb == njb - 1),
                                    )
                            # reciprocal of colsum -> broadcast tile in SBUF
                            for h in range(2):
                                with nc.allow_low_precision(reason="softmax denom recip to f32r"):
                                    recip = recip_pool.tile([1, 512], f32r, tag="recip", name=f"r{hp}_{ci}_{h}")
                                    nc.vector.reciprocal(recip, cs[h])
                                bc = bc_ps.tile([128, 512], f32, tag="bc", name=f"bc{hp}_{ci}_{h}")
                                nc.tensor.matmul(bc, ones_row, recip, start=True, stop=True)
                                bc_sb = bcs_pool.tile([128, 512], f32, tag="bcs", name=f"bs{hp}_{ci}_{h}")
                                nc.scalar.copy(out=bc_sb, in_=bc)
                                # evacuate scaled attn: head0 rows 0:64, head1 rows 64:128
                                rows = slice(0, 64) if h == 0 else slice(64, 128)
                                nc.vector.tensor_tensor(
                                    out=a8[hp][rows, ci * 512:(ci + 1) * 512],
                                    in0=p_at[h][rows, :],
                                    in1=bc_sb[rows, :],
                                    op=mybir.AluOpType.mult,
                                )

                # ---------------- phase C: output projection ----------------
                with (
                    tc.tile_pool(name="wop", bufs=1) as wo_pool,
                    tc.tile_pool(name="osb", bufs=3) as out_pool,
                    tc.tile_pool(name="psC", bufs=4, space="PSUM") as psC,
                ):
                    wo = [wo_pool.tile([128, D], f32r, tag=f"wo{i}", name=f"wo{i}") for i in range(NHP)]
                    with tc.tile_pool(name="wo_stage", bufs=2) as wst2:
                        for hp in range(NHP):
                            t = wst2.tile([128, D], f32, tag="wos", name=f"wos{hp}")
                            nc.sync.dma_start(out=t, in_=wo_d[hp * 128:(hp + 1) * 128, :])
                            nc.scalar.copy(out=wo[hp], in_=t)
                    for ib in range(NSB):
                        ot = out_pool.tile([128, D], f32, tag="ot", name=f"ot{ib}")
                        for eo in range(2):
                            ps = psC.tile([128, 512], f32, tag="psC", name=f"pC{ib}_{eo}")
                            for hp in range(NHP):
                                nc.tensor.matmul(
                                    ps,
                                    a8[hp][:, ib * 128:(ib + 1) * 128],
                                    wo[hp][:, eo * 512:(eo + 1) * 512],
                                    start=(hp == 0),
                                    stop=(hp == NHP - 1),
                                )
                            nc.vector.tensor_copy(ot[:, eo * 512:(eo + 1) * 512], ps)
                        nc.sync.dma_start(out=outp_d[ib * 128:(ib + 1) * 128, :], in_=ot)
    return nc


_NC_CACHE = {}


def _get_nc():
    if "nc" not in _NC_CACHE:
        nc = _build()
        _split_multi_waits(nc)
        _NC_CACHE["nc"] = nc
    return _NC_CACHE["nc"]


def _host_masks():
    m = np.zeros((4, 128, 512), np.float32)
    jl = np.arange(128)[:, None]
    il = np.arange(512)[None, :]
    for k in range(4):
        m[k] = (jl <= il - 128 * k).astype(np.float32)
    return m


def run_cores(x, Wq, Wk, Wv, Wo, trace=False):
    """Run the 8-core SPMD kernel; returns (per-core results list, BassKernelResults)."""
    nc = _get_nc()
    masks = _host_masks()
    in_maps = []
    for c in range(8):
        b = c // 2
        hh = c % 2
        sl = slice(hh * 512, (hh + 1) * 512)
        in_maps.append({
            "xt": np.ascontiguousarray(np.asarray(x)[b].T),
            "wq": np.ascontiguousarray(np.asarray(Wq)[:, sl]),
            "wk": np.ascontiguousarray(np.asarray(Wk)[:, sl]),
            "wv": np.ascontiguousarray(np.asarray(Wv)[:, sl]),
            "wo": np.ascontiguousarray(np.asarray(Wo)[sl, :]),
            "masks": masks,
        })
    res = run_bass_kernel_spmd(nc, in_maps, core_ids=list(range(8)), trace=trace)
    return res


def kernel(x, Wq, Wk, Wv, Wo, bo):
    x = np.asarray(x, np.float32)
    res = run_cores(x, Wq, Wk, Wv, Wo)
    out = np.empty((B, S, D), np.float32)
    probs = np.empty((B, H, S, S), np.float32)
    bo = np.asarray(bo, np.float32)
    for c in range(8):
        r = res.results[c]
        b = c // 2
        hh = c % 2
        if hh == 0:
            out[b] = r["outp"]
        else:
            out[b] += r["outp"]
        pt = r["probsT"]
        for hl in range(HPC):
            e = pt[hl]
            denom = e.sum(axis=0, dtype=np.float64)
            probs[b, hh * HPC + hl] = (e / denom[None, :].astype(np.float32)).T
    out += bo
    return (out, probs)
